# revision 4
# baseline (speedup 1.0000x reference)
"""Causal single-head attention (B=4, N=2048, E=1024, D=64) on 8 TRN2 NeuronCores.

Sharding: core i handles batch b = i//2, query rows with parity p = i%2
(rows p, p+2, p+4, ... of that batch -> 1024 local rows). The row-interleaved
split makes the causal workload identical on every core, so one SPMD program
serves all 8. K/V are used in full per core; Q is the strided half.

Per-core program (all matmuls in fp32r, full-rate on the PE):
  kT = Wk.T @ K.T   [64, 2048]     (PSUM-accumulated over 8 E-chunks)
  qT = Wq.T @ Q.T   [64, 1024]
  vT = Wv.T @ V.T   [64, 2048] -> PE-transpose -> v1 [2048, 65] (ones column)
  per q-block j (256 local cols = 512 original rows), k-chunks c<=4j+3:
    s^T  = kT_c.T @ qT_j        [128, 256]  (scores transposed)
    e    = exp(s^T / 8)         (ACT), causal mask via 0/1 multiply on DVE
    po  += v1_c.T @ e           [65, 256]   (row 64 = softmax denominator)
  out_j = transpose(po)[:, :64] * recip(transpose(po)[:, 64])
"""
import os
import sys

sys.path.insert(0, "/opt/trn_rl_repo")

import numpy as np

B, N, E, D = 4, 2048, 1024, 64
NQL = N // 2      # local q rows per core
QB = 256          # local q-block width (in qT columns)
KC = 128          # k chunk
EC = 128          # E chunk
NEC = E // EC     # 8
NKT = N // 512    # 4 col-tiles for kT/vT projections
PRECISION = os.environ.get("KERNEL_PRECISION", "f32r")

_NC_CACHE = {}


def _build_nc():
    from concourse import bacc, mybir, tile
    from concourse.masks import make_identity

    f32 = mybir.dt.float32
    f32r = mybir.dt.float32r if PRECISION == "f32r" else mybir.dt.float32
    AF = mybir.ActivationFunctionType

    nc = bacc.Bacc()
    KT = nc.dram_tensor("KT", [E, N], f32, kind="ExternalInput")
    QT = nc.dram_tensor("QT", [E, NQL], f32, kind="ExternalInput")
    VT = nc.dram_tensor("VT", [E, N], f32, kind="ExternalInput")
    WK = nc.dram_tensor("WK", [E, D], f32, kind="ExternalInput")
    WQ = nc.dram_tensor("WQ", [E, D], f32, kind="ExternalInput")
    WV = nc.dram_tensor("WV", [E, D], f32, kind="ExternalInput")
    MASK = nc.dram_tensor("MASK", [4, KC, QB], f32, kind="ExternalInput")
    OUT = nc.dram_tensor("OUT", [NQL, D], f32, kind="ExternalOutput")

    with tile.TileContext(nc) as tc:
        with (
            tc.tile_pool(name="consts", bufs=1) as consts,
            tc.tile_pool(name="kin", bufs=3) as kin,
            tc.tile_pool(name="qin", bufs=3) as qin,
            tc.tile_pool(name="vin", bufs=3) as vin,
            tc.tile_pool(name="proj", bufs=1) as proj,
            tc.tile_pool(name="expp", bufs=4) as expp,
            tc.tile_pool(name="epi", bufs=2) as epi,
        ):
            # ---- constants ----
            wk_all = consts.tile([EC, NEC, D], f32r, tag="wk")
            wq_all = consts.tile([EC, NEC, D], f32r, tag="wq")
            wv_all = consts.tile([EC, NEC, D], f32r, tag="wv")
            nc.sync.dma_start(wk_all[:], WK.rearrange("(c p) m -> p c m", p=EC).bitcast(f32r))
            nc.sync.dma_start(wq_all[:], WQ.rearrange("(c p) m -> p c m", p=EC).bitcast(f32r))
            nc.sync.dma_start(wv_all[:], WV.rearrange("(c p) m -> p c m", p=EC).bitcast(f32r))
            masks = consts.tile([KC, 4, QB], f32r, tag="mask")
            nc.sync.dma_start(masks[:], MASK.rearrange("m p q -> p m q").bitcast(f32r))
            ident = consts.tile([128, 128], f32, tag="ident")
            make_identity(nc, ident[:])

            kT_sb = proj.tile([D, N], f32r, tag="kT")
            qT_sb = proj.tile([D, NQL], f32r, tag="qT")
            vT_sb = proj.tile([D, N], f32, tag="vT")
            v1_sb = proj.tile([KC, N // KC, D + 1], f32r, tag="v1")

            # ---- k/q projections ----
            with tc.tile_pool(name="psKQ", bufs=1, space="PSUM") as psKQ:
                pk = [psKQ.tile([D, 512], f32, tag=f"pk{t}", name=f"pk{t}") for t in range(NKT)]
                pq = [psKQ.tile([D, 512], f32, tag=f"pq{t}", name=f"pq{t}") for t in range(2)]
                for c in range(NEC):
                    kt = kin.tile([EC, N], f32r, tag="kt")
                    nc.sync.dma_start(kt[:], KT[EC * c:EC * (c + 1), :].bitcast(f32r))
                    qt = qin.tile([EC, NQL], f32r, tag="qt")
                    nc.sync.dma_start(qt[:], QT[EC * c:EC * (c + 1), :].bitcast(f32r))
                    for t in range(NKT):
                        nc.tensor.matmul(pk[t][:], wk_all[:, c, :], kt[:, 512 * t:512 * (t + 1)],
                                         start=(c == 0), stop=(c == NEC - 1))
                    for t in range(2):
                        nc.tensor.matmul(pq[t][:], wq_all[:, c, :], qt[:, 512 * t:512 * (t + 1)],
                                         start=(c == 0), stop=(c == NEC - 1))
                for t in range(NKT):
                    nc.scalar.copy(kT_sb[:, 512 * t:512 * (t + 1)], pk[t][:])
                for t in range(2):
                    nc.scalar.copy(qT_sb[:, 512 * t:512 * (t + 1)], pq[t][:])

            # ---- v projection + transpose to [k, d] with ones column ----
            nc.gpsimd.memset(v1_sb[:].bitcast(f32), 1.0)
            with tc.tile_pool(name="psV", bufs=1, space="PSUM") as psV:
                pv = [psV.tile([D, 512], f32, tag=f"pv{t}", name=f"pv{t}") for t in range(NKT)]
                for c in range(NEC):
                    vt = vin.tile([EC, N], f32r, tag="vt")
                    nc.sync.dma_start(vt[:], VT[EC * c:EC * (c + 1), :].bitcast(f32r))
                    for t in range(NKT):
                        nc.tensor.matmul(pv[t][:], wv_all[:, c, :], vt[:, 512 * t:512 * (t + 1)],
                                         start=(c == 0), stop=(c == NEC - 1))
                for t in range(NKT):
                    nc.scalar.copy(vT_sb[:, 512 * t:512 * (t + 1)], pv[t][:])
                with tc.tile_pool(name="psVT", bufs=2, space="PSUM") as psVT:
                    for t in range(N // KC):
                        pvt = psVT.tile([KC, D], f32, tag="pvt")
                        nc.tensor.transpose(pvt[:], vT_sb[:, KC * t:KC * (t + 1)], ident[0:D, 0:D])
                        nc.scalar.copy(v1_sb[:, t, 0:D], pvt[:])

            # ---- attention ----
            with tc.tile_pool(name="psE", bufs=1, space="PSUM") as psE:
                for j in range(4):
                    po = psE.tile([D + 1, QB], f32, tag="po")
                    nch = 4 * j + 4
                    for c in range(nch):
                        ps = psE.tile([KC, QB], f32, tag="ps")
                        nc.tensor.matmul(ps[:], kT_sb[:, KC * c:KC * (c + 1)],
                                         qT_sb[:, QB * j:QB * (j + 1)], start=True, stop=True)
                        ex = expp.tile([KC, QB], f32r, tag="ex")
                        nc.scalar.activation(ex[:], ps[:], AF.Exp, scale=0.125)
                        if c >= 4 * j:
                            nc.vector.tensor_mul(ex[:], ex[:], masks[:, c - 4 * j, :])
                        nc.tensor.matmul(po[:], v1_sb[:, c, :], ex[:],
                                         start=(c == 0), stop=(c == nch - 1))
                    pot = epi.tile([D + 1, QB], f32, tag="pot")
                    nc.scalar.copy(pot[:], po[:])
                    ob = epi.tile([KC, 2, D], f32, tag="ob")
                    for h in range(2):
                        pq2 = psE.tile([KC, D + 1], f32, tag="pq2")
                        nc.tensor.transpose(pq2[:], pot[:, KC * h:KC * (h + 1)],
                                            ident[0:D + 1, 0:D + 1])
                        rcp = epi.tile([KC, 1], f32, tag="rcp")
                        nc.vector.reciprocal(rcp[:], pq2[:, D:D + 1])
                        nc.vector.tensor_scalar_mul(ob[:, h, :], pq2[:, 0:D], rcp[:])
                    nc.sync.dma_start(
                        OUT[QB * j:QB * (j + 1), :].rearrange("(h p) d -> p h d", p=KC),
                        ob[:])

    nc.finalize()
    return nc


def get_nc():
    if "nc" not in _NC_CACHE:
        _NC_CACHE["nc"] = _build_nc()
    return _NC_CACHE["nc"]


def shard_inputs(K, Q, V, Wk, Wq, Wv):
    K, Q, V = np.asarray(K), np.asarray(Q), np.asarray(V)
    Wk, Wq, Wv = (np.ascontiguousarray(np.asarray(a), dtype=np.float32) for a in (Wk, Wq, Wv))
    kk = np.arange(KC)
    qq = np.arange(QB)
    masks = {}
    for p in range(2):
        masks[p] = np.stack([
            (kk[:, None] + KC * m <= 2 * qq[None, :] + p).astype(np.float32)
            for m in range(4)
        ])
    in_maps = []
    for core in range(8):
        b, p = core // 2, core % 2
        in_maps.append({
            "KT": np.ascontiguousarray(K[b].T, dtype=np.float32),
            "QT": np.ascontiguousarray(Q[b].T[:, p::2], dtype=np.float32),
            "VT": np.ascontiguousarray(V[b].T, dtype=np.float32),
            "WK": Wk, "WQ": Wq, "WV": Wv,
            "MASK": masks[p],
        })
    return in_maps


def gather_outputs(outs):
    full = np.zeros((B, N, D), np.float32)
    for core in range(8):
        b, p = core // 2, core % 2
        full[b, p::2] = outs[core]
    return full


def kernel(K, Q, V, Wk, Wq, Wv):
    from concourse.bass_utils import run_bass_kernel_spmd

    in_maps = shard_inputs(K, Q, V, Wk, Wq, Wv)
    nc = get_nc()
    res = run_bass_kernel_spmd(nc, in_maps, list(range(8)))
    return gather_outputs([res.results[i]["OUT"] for i in range(8)])
